# revision 12
# baseline (speedup 1.0000x reference)
"""ALIGNN GNN message passing on 8 TRN2 NeuronCores (self-contained).

Sharding: core c owns nodes [128c,128c+128) and edges [1536c,1536c+1536)
(slot order e=v*12+k). Line-graph rows (e,k) live with owner(dst[e]) in the
AllToAll-recv layout (k-major over 8x256-padded recv slots, ZR=24576 rows).
Residual streams (x, m, z) kept feature-major bf16 in SBUF; row-major chains
(sigmoid/LN/silu) per tile with batched stats; segment sums + gathers as
one-hot bf16 matmuls; cross-core traffic via AllToAll collectives; row-major
-> feature-major conversion via DRAM-staged transpose-DMA.
"""
import sys
import types
import numpy as np

try:
    import antenv
    if not hasattr(antenv, "axon_hooks"):
        from trn_agent_boot.trn_boot import _ntff_profile_via_ctypes
        _mod = types.ModuleType("antenv.axon_hooks")
        _hook = _ntff_profile_via_ctypes("/opt/axon/libaxon_pjrt.so")
        _mod.get_axon_ntff_profile_hook = lambda: _hook
        _mod.set_axon_ntff_profile_hook = lambda h: None
        sys.modules["antenv.axon_hooks"] = _mod
        antenv.axon_hooks = _mod
except Exception:
    pass

from concourse import bass, bacc, tile, mybir, tile_utils
from concourse.bass_utils import run_bass_kernel_spmd
import ml_dtypes

BF16 = ml_dtypes.bfloat16
NCORE = 8
N, DEG, H = 1024, 12, 256
E = N * DEG
VPC = N // NCORE
EPC = E // NCORE              # 1536
SCH = EPC // 128              # 12
BLK = 256
RECV = NCORE * BLK            # 2048
RCH = RECV // 128             # 16
ZR = DEG * RECV               # 24576
NL = 12
EB, TB, EMB = 80, 40, 64
GLAYERS = (0, 2, 4, 6, 8, 9, 10, 11)
LGLNEXT = (0, 2, 4, 6)        # graph layers whose x_new rides next LG A2A
SEND_STRIDE = BLK + 64        # 320 rows of 512 bf16 per A2A block
EPS = 1e-5

f32 = mybir.dt.float32
bf16 = mybir.dt.bfloat16
i32 = mybir.dt.int32
AF = mybir.ActivationFunctionType
ALU = mybir.AluOpType


def _rbf(d, vmin, vmax, bins):
    centers = np.linspace(vmin, vmax, bins)
    gamma = (bins - 1.0) / (vmax - vmin)
    return np.exp(-gamma * (d[:, None] - centers[None, :]) ** 2)


def _prep(inp):
    src = np.asarray(inp["src"]).astype(np.int64)
    dst = np.asarray(inp["dst"]).astype(np.int64)
    r = np.asarray(inp["r"], np.float64)
    blen = np.linalg.norm(r, axis=1)
    r_on, r_cut = 7.5, 8.0
    r2, ron2, rc2 = blen * blen, r_on * r_on, r_cut * r_cut
    s = (rc2 - r2) ** 2 * (rc2 + 2.0 * r2 - 3.0 * ron2) / (rc2 - ron2) ** 3
    fcut = np.where(blen < r_on, 1.0, np.where(blen < r_cut, s, 0.0)).astype(np.float32)
    rbfe = _rbf(blen, 0.0, 8.0, EB).astype(np.float32)
    x0 = np.asarray(inp["atom_emb"])[np.asarray(inp["atomic_number"])]

    dcore = dst // VPC
    ecore = src // VPC
    sendpos = np.zeros(E, np.int64)
    recv_edge = -np.ones((NCORE, RECV), np.int64)
    for c in range(NCORE):
        for d in range(NCORE):
            es = np.nonzero((ecore == c) & (dcore == d))[0]
            assert len(es) <= BLK, f"A2A block overflow: {len(es)} > {BLK}"
            sendpos[es] = d * SEND_STRIDE + np.arange(len(es))
            recv_edge[d, c * BLK:c * BLK + len(es)] = es

    gW = np.asarray(inp["gate_W"]); gb = np.asarray(inp["gate_b"])
    uW = np.asarray(inp["upd_W"]); ub = np.asarray(inp["upd_b"])
    WX = np.concatenate([gW[:, 0], gW[:, 1], uW[:, 1], uW[:, 0]], axis=2)
    xtb = np.zeros((NL, H * 4), np.float32)
    xtb[:, 0:H] = gb[:, 0] + gb[:, 1] + gb[:, 2]
    xtb[:, 2 * H:3 * H] = ub[:, 1]
    xtb[:, 3 * H:4 * H] = ub[:, 0]
    WAB = np.concatenate([gW[:, 0], uW[:, 1]], axis=2)
    lgsb = np.zeros((NL, 512), np.float32)
    lgsb[:, 0:H] = gb[:, 0] + gb[:, 1] + gb[:, 2]
    lgsb[:, H:2 * H] = ub[:, 1]
    lng = np.asarray(inp["ln_g"]); lnb = np.asarray(inp["ln_b"])

    # kv-major local order: position p=k*128+v  <->  slot v*DEG+k
    kvperm = np.array([(p % 128) * DEG + p // 128 for p in range(EPC)], np.int64)
    srcoh = np.zeros((128, SCH * 128), np.float32)
    for p in range(EPC):
        srcoh[p % 128, p] = 1.0

    def bc(a, shape):
        return np.ascontiguousarray(np.broadcast_to(a, shape))

    shared = {
        "x0_fm": np.ascontiguousarray(x0.T.astype(BF16)),
        "srcoh": srcoh.astype(BF16),
        "wx": np.ascontiguousarray(WX.astype(BF16)),
        "xtb": bc(xtb[:, None, :], (NL, 128, H * 4)).astype(BF16),
        "wab": np.ascontiguousarray(WAB.astype(BF16)),
        "lgsb": bc(lgsb[:, None, :], (NL, 128, 512)).astype(BF16),
        "gw1": np.ascontiguousarray(gW[:, 1].astype(BF16)),
        "gw2": np.ascontiguousarray(gW[:, 2].astype(BF16)),
        "uw0": np.ascontiguousarray(uW[:, 0].astype(BF16)),
        "ub0b": bc(ub[:, 0][:, None, :], (NL, 128, H)).astype(np.float32),
        "lng": bc(lng[:, :, None, :], (NL, 2, 128, H)).astype(BF16),
        "lnbb": bc(lnb[:, :, None, :], (NL, 2, 128, H)).astype(BF16),
        "ew1": np.asarray(inp["edge_W1"]).astype(BF16),
        "ew2": np.asarray(inp["edge_W2"]).astype(BF16),
        "aw1": np.asarray(inp["ang_W1"]).astype(BF16),
        "aw2": np.asarray(inp["ang_W2"]).astype(BF16),
        "e1c": bc(np.stack([np.asarray(inp["edge_b1"]), np.asarray(inp["edge_g1"]),
                            np.asarray(inp["edge_beta1"])])[:, None, :], (3, 128, EMB)).astype(BF16),
        "e2c": bc(np.stack([np.asarray(inp["edge_b2"]), np.asarray(inp["edge_g2"]),
                            np.asarray(inp["edge_beta2"])])[:, None, :], (3, 128, H)).astype(BF16),
        "a1c": bc(np.stack([np.asarray(inp["ang_b1"]), np.asarray(inp["ang_g1"]),
                            np.asarray(inp["ang_beta1"])])[:, None, :], (3, 128, EMB)).astype(BF16),
        "a2c": bc(np.stack([np.asarray(inp["ang_b2"]), np.asarray(inp["ang_g2"]),
                            np.asarray(inp["ang_beta2"])])[:, None, :], (3, 128, H)).astype(BF16),
        "fcw": np.asarray(inp["fc_W"]).astype(np.float32),
        "fcb": np.asarray(inp["fc_b"]).reshape(1, 1).astype(np.float32),
    }

    maps = []
    for c in range(NCORE):
        e0 = c * EPC
        eids = e0 + kvperm
        re = recv_edge[c]
        valid = re >= 0
        rg = np.zeros((RECV, 128), np.float32)
        rg[valid, dst[re[valid]] - c * VPC] = 1.0
        incoh = np.zeros((128, RECV), np.float32)
        incoh[dst[re[valid]] - c * VPC, np.nonzero(valid)[0]] = 1.0
        ownoh = np.zeros((128, NCORE, 128), np.float32)
        ownoh[:, c, :] = np.eye(128)
        cosr = np.zeros((DEG, RECV), np.float64)
        ev = re[valid]
        r1 = -r[ev]
        n1 = np.linalg.norm(r1, axis=1)
        for k in range(DEG):
            f = dst[ev] * DEG + k
            r2v = r[f]
            cos = (r1 * r2v).sum(1) / (n1 * np.linalg.norm(r2v, axis=1))
            cosr[k, valid] = np.clip(cos, -1.0, 1.0)
        rbfa = _rbf(cosr.reshape(-1), -1.0, 1.0, TB).astype(np.float32)
        m = dict(shared)
        m.update({
            "x0_own": np.ascontiguousarray(x0[c * VPC:(c + 1) * VPC].astype(BF16)),
            "rbfe_fm": np.ascontiguousarray(rbfe[eids].T.astype(BF16)),
            "rbfa_fm": np.ascontiguousarray(rbfa.T.astype(BF16)),
            "fcutm": np.ascontiguousarray(fcut[eids].reshape(SCH, 128).T.astype(np.float32)),
            "dstidx": np.ascontiguousarray(dst[eids].reshape(SCH, 128).T.astype(np.int32)),
            "sendpos": np.ascontiguousarray(sendpos[eids].reshape(SCH, 128).T.astype(np.int32)),
            "rg": rg.astype(BF16),
            "incoh": incoh.astype(BF16),
            "ownoh": np.ascontiguousarray(ownoh.astype(BF16)),
        })
        maps.append(m)
    return maps


def _build():
    pass
    nc = bacc.Bacc("TRN2", target_bir_lowering=False, debug=False,
                   num_devices=NCORE)
    D = {}
    def din(name, shape, dt):
        D[name] = nc.dram_tensor(name, list(shape), dt, kind="ExternalInput")
    din("x0_fm", (H, N), bf16); din("x0_own", (VPC, H), bf16)
    din("rbfe_fm", (EB, EPC), bf16); din("rbfa_fm", (TB, ZR), bf16)
    din("fcutm", (128, SCH), f32); din("dstidx", (128, SCH), i32)
    din("sendpos", (128, SCH), i32)
    din("srcoh", (128, EPC), bf16); din("rg", (RECV, 128), bf16)
    din("incoh", (128, RECV), bf16); din("ownoh", (128, NCORE, 128), bf16)
    din("wx", (NL, H, 4 * H), bf16); din("xtb", (NL, 128, 4 * H), bf16)
    din("wab", (NL, H, 512), bf16); din("lgsb", (NL, 128, 512), bf16)
    din("gw1", (NL, H, H), bf16); din("gw2", (NL, H, H), bf16)
    din("uw0", (NL, H, H), bf16); din("ub0b", (NL, 128, H), f32)
    din("lng", (NL, 2, 128, H), bf16); din("lnbb", (NL, 2, 128, H), bf16)
    din("ew1", (EB, EMB), bf16); din("ew2", (EMB, H), bf16)
    din("aw1", (TB, EMB), bf16); din("aw2", (EMB, H), bf16)
    din("e1c", (3, 128, EMB), bf16); din("e2c", (3, 128, H), bf16)
    din("a1c", (3, 128, EMB), bf16); din("a2c", (3, 128, H), bf16)
    din("fcw", (H, 1), f32); din("fcb", (1, 1), f32)
    out = nc.dram_tensor("out", [1, 1], f32, kind="ExternalOutput")

    # DRAM scratch (raw internal tensors; offset 0 for indirect DMA)
    sendb = nc.dram_tensor("sendb", [NCORE * SEND_STRIDE, 512], bf16)
    recvb = nc.dram_tensor("recvb", [NCORE * SEND_STRIDE, 512], bf16)
    sendx = nc.dram_tensor("sendx", [N, H], bf16)
    recvx = nc.dram_tensor("recvx", [N, H], bf16)
    bxst = nc.dram_tensor("bxst", [N, H], bf16)
    szst = nc.dram_tensor("szst", [ZR, H], bf16)
    smst = nc.dram_tensor("smst", [EPC, H], bf16)
    s1st = nc.dram_tensor("s1st", [ZR, 128], bf16)

    with tile.TileContext(nc) as tc:
        with tc.tile_pool(name="per", bufs=1) as per, \
             tc.tile_pool(name="big", bufs=1) as big, \
             tc.tile_pool(name="sb", bufs=1) as sb, \
             tc.tile_pool(name="pw", bufs=2, space="PSUM") as pwp, \
             tc.tile_pool(name="pn", bufs=2, space="PSUM") as pnp, \
             tc.tile_pool(name="pH", bufs=1, space="PSUM") as pHp, \
             tc.tile_pool(name="pO", bufs=1, space="PSUM") as pOp:

            z_fm = per.tile([128, 2, ZR], bf16)
            m_fm = per.tile([128, 2, EPC], bf16)
            x_fm = per.tile([128, 2, N], bf16)
            x_own = per.tile([128, H], bf16)
            rg_t = per.tile([128, RCH, 128], bf16)
            fcut_t = per.tile([128, SCH], f32)
            dsti_t = per.tile([128, SCH], i32)
            sp_t = per.tile([128, SCH], i32)

            nc.sync.dma_start(out=x_fm[:, 0, :], in_=D["x0_fm"][0:128, :])
            nc.sync.dma_start(out=x_fm[:, 1, :], in_=D["x0_fm"][128:256, :])
            nc.sync.dma_start(out=x_own[:], in_=D["x0_own"][:])
            nc.sync.dma_start(out=rg_t[:],
                              in_=D["rg"][:].rearrange("(t p) v -> p t v", p=128))
            zt = sb.tile([128, 512], bf16, tag="ab")
            nc.vector.memset(zt[:], 0.0)
            for j in range(NCORE * SEND_STRIDE // 128):
                nc.sync.dma_start(out=sendb[j * 128:(j + 1) * 128, :], in_=zt[:])
            nc.sync.dma_start(out=fcut_t[:], in_=D["fcutm"][:])
            nc.sync.dma_start(out=dsti_t[:], in_=D["dstidx"][:])
            nc.sync.dma_start(out=sp_t[:], in_=D["sendpos"][:])

            def ln_silu(win, T, lg_t, lnb_t, ob):
                """LN+affine+silu: win [128,T,W] -> ob [128,T,W] (bf16)."""
                W = win.shape[-1]
                st0 = sb.tile([128, RCH, 1], f32, tag="st0")
                st1 = sb.tile([128, RCH, 1], f32, tag="st1")
                nc.scalar.activation(ob, win, AF.Square)
                nc.vector.tensor_reduce(st0[:, :T, :], win, mybir.AxisListType.X, ALU.add)
                nc.vector.tensor_reduce(st1[:, :T, :], ob, mybir.AxisListType.X, ALU.add)
                mu = sb.tile([128, RCH], f32, tag="lnmu")
                va = sb.tile([128, RCH], f32, tag="lnva")
                m2 = sb.tile([128, RCH], f32, tag="lnm2")
                sd = sb.tile([128, RCH], f32, tag="lnsd")
                nc.vector.tensor_scalar(mu[:, :T], st0[:, :T, 0].opt(), 1.0 / W, None, ALU.mult)
                nc.vector.tensor_scalar(va[:, :T], st1[:, :T, 0].opt(), 1.0 / W, None, ALU.mult)
                nc.vector.tensor_tensor(m2[:, :T], mu[:, :T], mu[:, :T], ALU.mult)
                nc.vector.tensor_tensor(va[:, :T], va[:, :T], m2[:, :T], ALU.subtract)
                nc.vector.tensor_scalar(va[:, :T], va[:, :T], EPS, None, ALU.add)
                nc.scalar.activation(sd[:, :T], va[:, :T], AF.Sqrt)
                nc.vector.reciprocal(va[:, :T], sd[:, :T])
                for t in range(T):
                    nc.vector.tensor_scalar(ob[:, t, :], win[:, t, :],
                                            mu[:, t:t + 1], va[:, t:t + 1],
                                            ALU.subtract, ALU.mult)
                gb_ = lg_t.opt().unsqueeze(1).to_broadcast((128, T, W))
                bb_ = lnb_t.opt().unsqueeze(1).to_broadcast((128, T, W))
                nc.vector.tensor_tensor(ob, ob, gb_, ALU.mult)
                nc.vector.tensor_tensor(ob, ob, bb_, ALU.add)
                nc.scalar.activation(ob, ob, AF.Silu)

            # ---------------- init MLPs (y0 -> m_fm, z0 -> z_fm) ----------------
            def mlp_init(rbf_d, nbins, w1_d, w2_d, c1_d, c2_d, rows, st1_d, out_d):
                nch = rows // 128
                w1t = sb.tile([EB, EMB], bf16, tag="w1t")
                w2t = sb.tile([EMB, H], bf16, tag="w2t")
                nc.sync.dma_start(out=w1t[:nbins, :], in_=w1_d[:])
                nc.sync.dma_start(out=w2t[:], in_=w2_d[:])
                c1 = sb.tile([128, 3, EMB], bf16, tag="c1")
                c2 = sb.tile([128, 3, H], bf16, tag="c2")
                nc.sync.dma_start(out=c1[:], in_=c1_d[:].rearrange("c p e -> p c e"))
                nc.sync.dma_start(out=c2[:], in_=c2_d[:].rearrange("c p e -> p c e"))
                TT = 8
                for b in range(0, nch, TT):
                    tb = min(TT, nch - b)
                    h1 = sb.tile([128, TT, EMB], bf16, tag="mz")
                    for t in range(tb):
                        p1 = pnp.tile([128, H], f32, tag="pn")
                        rb = sb.tile([EB, 128], bf16, tag="xt")
                        nc.sync.dma_start(out=rb[:nbins, :],
                                          in_=rbf_d[:, (b + t) * 128:(b + t + 1) * 128])
                        nc.tensor.matmul(p1[:, :EMB], rb[:nbins, :], w1t[:nbins, :],
                                         start=True, stop=True)
                        nc.vector.tensor_tensor(h1[:, t, :], p1[:, :EMB], c1[:, 0, :], ALU.add)
                    ob = sb.tile([128, TT, EMB], bf16, tag="sgb")
                    ln_silu(h1[:, :tb, :], tb, c1[:, 1, :], c1[:, 2, :], ob[:, :tb, :])
                    nc.sync.dma_start(
                        out=st1_d[b * 128:(b + tb) * 128, 0:EMB].rearrange(
                            "(t p) e -> p t e", p=128),
                        in_=ob[:, :tb, :])
                qs = rows // 8
                for q in range(8):
                    s1f = big.tile([128, ZR // 8], bf16, tag="s1f")
                    nc.sync.dma_start_transpose(s1f[:, :qs], st1_d[q * qs:(q + 1) * qs, :])
                    for b in range(0, qs // 128, TT):
                        tb = min(TT, qs // 128 - b)
                        h2 = sb.tile([128, TT, H], bf16, tag="mz")
                        for t in range(tb):
                            p2 = pnp.tile([128, H], f32, tag="pn")
                            nc.tensor.matmul(p2[:], s1f[:EMB, (b + t) * 128:(b + t + 1) * 128],
                                             w2t[:], start=True, stop=True)
                            nc.vector.tensor_tensor(h2[:, t, :], p2[:], c2[:, 0, :], ALU.add)
                        ob2 = sb.tile([128, TT, H], bf16, tag="sgb")
                        ln_silu(h2[:, :tb, :], tb, c2[:, 1, :], c2[:, 2, :], ob2[:, :tb, :])
                        nc.sync.dma_start(
                            out=out_d[q * qs + b * 128:q * qs + (b + tb) * 128, :].rearrange(
                                "(t p) e -> p t e", p=128),
                            in_=ob2[:, :tb, :])

            mlp_init(D["rbfe_fm"], EB, D["ew1"], D["ew2"], D["e1c"], D["e2c"],
                     EPC, s1st, smst)
            for ch in range(2):
                nc.sync.dma_start_transpose(m_fm[:, ch, :],
                                            smst[:, ch * 128:(ch + 1) * 128])
            mlp_init(D["rbfa_fm"], TB, D["aw1"], D["aw2"], D["a1c"], D["a2c"],
                     ZR, s1st, szst)
            for ch in range(2):
                for hf in range(8):
                    nc.sync.dma_start_transpose(
                        z_fm[:, ch, hf * (ZR // 8):(hf + 1) * (ZR // 8)],
                        szst[hf * (ZR // 8):(hf + 1) * (ZR // 8),
                             ch * 128:(ch + 1) * 128])

            def recv_segsum_h(rhs3):
                """segsum over 16 chunks of rhs3 [128,16,512] -> h [128,256] bf16"""
                Hp = pHp.tile([128, 512], f32, tag="Hp")
                for t in range(RCH):
                    nc.tensor.matmul(Hp[:], rg_t[:, t, :], rhs3[:, t, :],
                                     start=(t == 0), stop=(t == RCH - 1))
                den = sb.tile([128, H], f32, tag="den")
                nc.vector.tensor_scalar(den[:], Hp[:, H:2 * H], 1e-6, None, ALU.add)
                rec = sb.tile([128, H], f32, tag="rec")
                nc.vector.reciprocal(rec[:], den[:])
                hh = sb.tile([128, H], bf16, tag="hh")
                nc.vector.tensor_tensor(hh[:], Hp[:, 0:H], rec[:], ALU.mult)
                return hh

            for l in range(NL):
                gw2 = sb.tile([128, 2, H], bf16, tag="gw2l")
                nc.sync.dma_start(out=gw2[:], in_=D["gw2"][l].rearrange("(c p) f -> p c f", p=128))
                uw0 = sb.tile([128, 2, H], bf16, tag="uw0l")
                nc.sync.dma_start(out=uw0[:], in_=D["uw0"][l].rearrange("(c p) f -> p c f", p=128))
                lg0 = sb.tile([128, H], bf16, tag="lg0")
                lb0 = sb.tile([128, H], bf16, tag="lb0")
                lg1 = sb.tile([128, H], bf16, tag="lg1")
                lb1 = sb.tile([128, H], bf16, tag="lb1")
                nc.sync.dma_start(out=lg0[:], in_=D["lng"][l, 0])
                nc.sync.dma_start(out=lb0[:], in_=D["lnbb"][l, 0])
                nc.sync.dma_start(out=lg1[:], in_=D["lng"][l, 1])
                nc.sync.dma_start(out=lb1[:], in_=D["lnbb"][l, 1])

                if l in GLAYERS:
                    wx = sb.tile([128, 2, 4 * H], bf16, tag="wxl")
                    nc.sync.dma_start(out=wx[:], in_=D["wx"][l].rearrange("(c p) f -> p c f", p=128))
                    xb = sb.tile([128, 4 * H], bf16, tag="xbl")
                    nc.sync.dma_start(out=xb[:], in_=D["xtb"][l])
                    srcoh_t = big.tile([128, SCH, 128], bf16, tag="rt")
                    nc.sync.dma_start(out=srcoh_t[:],
                                      in_=D["srcoh"][:].rearrange("p (s j) -> p s j", s=SCH))
                    ownoh_t = big.tile([128, NCORE, 128], bf16, tag="sfm")
                    nc.sync.dma_start(out=ownoh_t[:], in_=D["ownoh"][:])
                    po = pOp.tile([128, 4 * H], f32, tag="pown")
                    for vch in range(8):
                        xt = sb.tile([128, 4 * H], bf16, tag="xt")
                        for hf in range(2):
                            pxt = pwp.tile([128, 512], f32, tag="pw")
                            for ch in range(2):
                                nc.tensor.matmul(pxt[:], x_fm[:, ch, vch * 128:(vch + 1) * 128],
                                                 wx[:, ch, hf * 512:(hf + 1) * 512],
                                                 start=(ch == 0), stop=(ch == 1))
                            nc.vector.tensor_tensor(xt[:, hf * 512:(hf + 1) * 512], pxt[:],
                                                    xb[:, hf * 512:(hf + 1) * 512], ALU.add)
                        nc.sync.dma_start(out=bxst[vch * 128:(vch + 1) * 128, :],
                                          in_=xt[:, H:2 * H])
                        for hf in range(2):
                            nc.tensor.matmul(po[:, hf * 512:(hf + 1) * 512],
                                             ownoh_t[:, vch, :],
                                             xt[:, hf * 512:(hf + 1) * 512],
                                             start=(vch == 0), stop=(vch == 7))
                    ownsel = sb.tile([128, 4 * H], bf16, tag="ownsel")
                    nc.scalar.activation(ownsel[:], po[:], AF.Copy)
                    bxg = sb.tile([128, SCH, H], bf16, tag="bxg")
                    for s2 in range(SCH):
                        nc.gpsimd.indirect_dma_start(
                            out=bxg[:, s2, :], out_offset=None, in_=bxst[:],
                            in_offset=bass.IndirectOffsetOnAxis(ap=dsti_t[:, s2:s2 + 1], axis=0))
                    mbuf = sb.tile([128, SCH, 512], bf16, tag="mbuf")
                    sgbuf = sb.tile([128, RCH, 512], bf16, tag="sgb")
                    for s2 in range(SCH):
                        pm = pwp.tile([128, 512], f32, tag="pw")
                        nc.tensor.matmul(
                            pm[:], srcoh_t[:, s2, :],
                            ownsel[:].rearrange("p (b f) -> p b f", b=2)[:, :, 0:H],
                            start=True, stop=False)
                        for ch in range(2):
                            nc.tensor.matmul(pm[:, 0:H], m_fm[:, ch, s2 * 128:(s2 + 1) * 128],
                                             gw2[:, ch, :], start=False, stop=(ch == 1))
                        nc.vector.tensor_tensor(mbuf[:, s2, 0:H], pm[:, 0:H],
                                                bxg[:, s2, :], ALU.add)
                        nc.scalar.activation(mbuf[:, s2, H:2 * H], pm[:, H:2 * H], AF.Copy)
                    nc.scalar.activation(sgbuf[:, :SCH, H:2 * H], mbuf[:, :, 0:H], AF.Sigmoid)
                    for s2 in range(SCH):
                        nc.vector.tensor_scalar(sgbuf[:, s2, H:2 * H], sgbuf[:, s2, H:2 * H],
                                                fcut_t[:, s2:s2 + 1], None, ALU.mult)
                    nc.vector.tensor_tensor(sgbuf[:, :SCH, 0:H], sgbuf[:, :SCH, H:2 * H],
                                            mbuf[:, :, H:2 * H], ALU.mult)
                    for s2 in range(SCH):
                        nc.gpsimd.indirect_dma_start(
                            out=sendb[:], out_offset=bass.IndirectOffsetOnAxis(
                                ap=sp_t[:, s2:s2 + 1], axis=0),
                            in_=sgbuf[:, s2, :], in_offset=None)
                    nc.gpsimd.collective_compute(
                        "AllToAll", ALU.bypass, replica_groups=[list(range(NCORE))],
                        ins=[sendb.ap().opt()], outs=[recvb.ap().opt()])
                    rt = big.tile([128, RCH, 512], bf16, tag="rt")
                    for b in range(NCORE):
                        for hb in range(2):
                            nc.sync.dma_start(
                                out=rt[:, b * 2 + hb, :],
                                in_=recvb[b * SEND_STRIDE + hb * 128:
                                          b * SEND_STRIDE + (hb + 1) * 128, :])
                    hh = recv_segsum_h(rt)
                    wxt = sb.tile([128, 1, H], bf16, tag="wxt")
                    nc.vector.tensor_tensor(wxt[:, 0, :], ownsel[:, 3 * H:4 * H], hh[:], ALU.add)
                    sx = sb.tile([128, 1, H], bf16, tag="sx")
                    ln_silu(wxt[:], 1, lg0[:], lb0[:], sx[:])
                    xn = sb.tile([128, H], bf16, tag="xn")
                    nc.vector.tensor_tensor(xn[:], x_own[:], sx[:, 0, :], ALU.add)
                    nc.vector.tensor_copy(x_own[:], xn[:])
                    # y chain
                    sy = sb.tile([128, SCH, H], bf16, tag="sgb")
                    ln_silu(mbuf[:, :, 0:H], SCH, lg1[:], lb1[:], sy[:])
                    nc.sync.dma_start(
                        out=smst[:].rearrange("(t p) e -> p t e", p=128), in_=sy[:])
                    for ch in range(2):
                        sfm = big.tile([128, EPC], bf16, tag="sfm")
                        nc.sync.dma_start_transpose(sfm[:], smst[:, ch * 128:(ch + 1) * 128])
                        nc.vector.tensor_tensor(m_fm[:, ch, :], m_fm[:, ch, :], sfm[:], ALU.add)
                    # x dissemination
                    if l in LGLNEXT:
                        for d in range(NCORE):
                            xrv = sendb[d * SEND_STRIDE + BLK:d * SEND_STRIDE + BLK + 64, :]
                            nc.sync.dma_start(
                                out=xrv.rearrange("r f -> (r f)").rearrange(
                                    "(n e) -> n e", e=H),
                                in_=xn[:])
                    else:
                        for d in range(NCORE):
                            nc.sync.dma_start(out=sendx[d * VPC:(d + 1) * VPC, :], in_=xn[:])
                        nc.gpsimd.collective_compute(
                            "AllToAll", ALU.bypass, replica_groups=[list(range(NCORE))],
                            ins=[sendx.ap().opt()], outs=[recvx.ap().opt()])
                        for ch in range(2):
                            nc.sync.dma_start_transpose(
                                x_fm[:, ch, :], recvx[:, ch * 128:(ch + 1) * 128])
                else:
                    # ---------------- LG layer ----------------
                    wab = sb.tile([128, 2, 512], bf16, tag="wabl")
                    nc.sync.dma_start(out=wab[:], in_=D["wab"][l].rearrange("(c p) f -> p c f", p=128))
                    gw1 = sb.tile([128, 2, H], bf16, tag="gw1l")
                    nc.sync.dma_start(out=gw1[:], in_=D["gw1"][l].rearrange("(c p) f -> p c f", p=128))
                    lsb = sb.tile([128, 512], bf16, tag="lsbl")
                    nc.sync.dma_start(out=lsb[:], in_=D["lgsb"][l])
                    ub0 = sb.tile([128, H], f32, tag="ub0l")
                    nc.sync.dma_start(out=ub0[:], in_=D["ub0b"][l])
                    incoh_t = sb.tile([128, RECV], bf16, tag="mbuf")
                    nc.sync.dma_start(out=incoh_t[:], in_=D["incoh"][:])
                    for s2 in range(SCH):
                        pa = pwp.tile([128, 512], f32, tag="pw")
                        for ch in range(2):
                            nc.tensor.matmul(pa[:], m_fm[:, ch, s2 * 128:(s2 + 1) * 128],
                                             wab[:, ch, :], start=(ch == 0), stop=(ch == 1))
                        ab = sb.tile([128, 512], bf16, tag="ab")
                        nc.vector.tensor_tensor(ab[:], pa[:], lsb[:], ALU.add)
                        nc.gpsimd.indirect_dma_start(
                            out=sendb[:], out_offset=bass.IndirectOffsetOnAxis(
                                ap=sp_t[:, s2:s2 + 1], axis=0),
                            in_=ab[:], in_offset=None)
                    nc.gpsimd.collective_compute(
                        "AllToAll", ALU.bypass, replica_groups=[list(range(NCORE))],
                        ins=[sendb.ap().opt()], outs=[recvb.ap().opt()])
                    for b in range(NCORE):
                        xr = recvb[b * SEND_STRIDE + BLK:b * SEND_STRIDE + BLK + 64, :]
                        xrv = xr.rearrange("r f -> (r f)").rearrange("(n e) -> n e", e=H)
                        for ch in range(2):
                            nc.sync.dma_start_transpose(
                                x_fm[:, ch, b * 128:(b + 1) * 128],
                                xrv[:, ch * 128:(ch + 1) * 128])
                    bkv = sb.tile([128, DEG, H], bf16, tag="bxg")
                    for k in range(DEG):
                        pb = pnp.tile([128, H], f32, tag="pn")
                        for ch in range(2):
                            nc.tensor.matmul(
                                pb[:],
                                m_fm[:, ch, k * 128:(k + 1) * 128],
                                gw1[:, ch, :], start=(ch == 0), stop=(ch == 1))
                        nc.scalar.activation(bkv[:, k, :], pb[:], AF.Copy)
                    rt = big.tile([128, RCH, 512], bf16, tag="rt")
                    for b in range(NCORE):
                        for hb in range(2):
                            nc.sync.dma_start(
                                out=rt[:, b * 2 + hb, :],
                                in_=recvb[b * SEND_STRIDE + hb * 128:
                                          b * SEND_STRIDE + (hb + 1) * 128, :])
                    for k in range(DEG):
                        mz = sb.tile([128, RCH, H], bf16, tag="mz")
                        sgz = sb.tile([128, RCH, 512], bf16, tag="sgb")
                        for t in range(RCH):
                            pz = pnp.tile([128, H], f32, tag="pn")
                            for ch in range(2):
                                nc.tensor.matmul(
                                    pz[:],
                                    z_fm[:, ch, k * RECV + t * 128:k * RECV + (t + 1) * 128],
                                    gw2[:, ch, :], start=(ch == 0), stop=False)
                            nc.tensor.matmul(pz[:], incoh_t[:, t * 128:(t + 1) * 128],
                                             bkv[:, k, :], start=False, stop=True)
                            nc.vector.tensor_tensor(mz[:, t, :], pz[:], rt[:, t, 0:H], ALU.add)
                        nc.scalar.activation(sgz[:, :, H:2 * H], mz[:], AF.Sigmoid)
                        nc.vector.tensor_tensor(sgz[:, :, 0:H], sgz[:, :, H:2 * H],
                                                rt[:, :, H:2 * H], ALU.mult)
                        hh = recv_segsum_h(sgz)
                        hk = sb.tile([128, H], f32, tag="hk")
                        nc.vector.tensor_tensor(hk[:], hh[:], ub0[:], ALU.add)
                        pu = pnp.tile([128, H], f32, tag="pn")
                        for ch in range(2):
                            nc.tensor.matmul(
                                pu[:],
                                m_fm[:, ch, k * 128:(k + 1) * 128],
                                uw0[:, ch, :], start=(ch == 0), stop=(ch == 1))
                        wm = sb.tile([128, 1, H], bf16, tag="wxt")
                        nc.vector.tensor_tensor(wm[:, 0, :], pu[:], hk[:], ALU.add)
                        sm = sb.tile([128, 1, H], bf16, tag="sx")
                        ln_silu(wm[:], 1, lg0[:], lb0[:], sm[:])
                        nc.sync.dma_start(out=smst[k * 128:(k + 1) * 128, :], in_=sm[:, 0, :])
                        sz = sb.tile([128, RCH, H], bf16, tag="sgb")
                        ln_silu(mz[:], RCH, lg1[:], lb1[:], sz[:])
                        nc.sync.dma_start(
                            out=szst[k * RECV:(k + 1) * RECV, :].rearrange(
                                "(t p) e -> p t e", p=128),
                            in_=sz[:])
                    for ch in range(2):
                        smf = big.tile([128, EPC], bf16, tag="sfm")
                        nc.sync.dma_start_transpose(smf[:], smst[:, ch * 128:(ch + 1) * 128])
                        nc.vector.tensor_tensor(m_fm[:, ch, :], m_fm[:, ch, :],
                                                smf[:], ALU.add)
                    for ch in range(2):
                        for hf in range(8):
                            szf = big.tile([128, ZR // 8], bf16, tag="s1f")
                            nc.sync.dma_start_transpose(
                                szf[:], szst[hf * (ZR // 8):(hf + 1) * (ZR // 8),
                                             ch * 128:(ch + 1) * 128])
                            zv = z_fm[:, ch, hf * (ZR // 8):(hf + 1) * (ZR // 8)]
                            nc.vector.tensor_tensor(zv, zv, szf[:], ALU.add)

            # ---------------- energy ----------------
            xm = sb.tile([128, 2], f32, tag="xm")
            nc.vector.tensor_reduce(xm[:], x_fm[:], mybir.AxisListType.X, ALU.add)
            fcw_t = sb.tile([128, 2], f32, tag="fcwt")
            nc.sync.dma_start(out=fcw_t[:],
                              in_=D["fcw"][:].rearrange("(c p) o -> p (c o)", p=128))
            pe = pnp.tile([1, 1], f32, tag="pn")
            for ch in range(2):
                nc.tensor.matmul(pe[:], xm[:, ch:ch + 1], fcw_t[:, ch:ch + 1],
                                 start=(ch == 0), stop=(ch == 1))
            fcb_t = sb.tile([1, 1], f32, tag="fcbt")
            nc.sync.dma_start(out=fcb_t[:], in_=D["fcb"][:])
            eo = sb.tile([1, 1], f32, tag="eo")
            nc.vector.tensor_scalar(eo[:], pe[:], 1.0 / N, None, ALU.mult)
            nc.vector.tensor_tensor(eo[:], eo[:], fcb_t[:], ALU.add)
            nc.sync.dma_start(out=out[:], in_=eo[:])
    nc.compile()
    return nc


_NC_CACHE = None


def kernel(**inputs):
    global _NC_CACHE
    maps = _prep(inputs)
    if _NC_CACHE is None:
        _NC_CACHE = _build()
    res = run_bass_kernel_spmd(_NC_CACHE, maps, core_ids=list(range(NCORE)))
    return np.asarray(res.results[0]["out"], np.float32).reshape(())
